# revision 1
# baseline (speedup 1.0000x reference)
"""HDMR network kernel for Trainium2 (Bass/Tile), 8-core batch-parallel.

The reference computes 92 small MLPs (8 first-order, 28 pair, 56 triple
sub-networks, each d_in -> 128 -> 128 -> 128 -> 1 with sigmoid) and
combines them with telescoping subtractions.  Those subtractions are a
fixed linear map, so the final output collapses to

    final[b] = c_f0 * f0 + sum_n c_n * g_n(x[b])

with integer coefficients c_n derived host-side by exact linear
expansion.  c_n is folded into each net's output-layer weights, so the
device just runs the 92 MLPs and accumulates weighted scalar outputs
into PSUM.

All matmuls use float32r (fp32 storage, FP22 multiply at full PE rate);
bf16 in the output layer loses too much precision because the folded
coefficients reach |c|=120.

Sharding: batch 8192 -> 1024 per core on 8 cores, weights replicated,
no collectives.
"""

import itertools
from contextlib import ExitStack

import numpy as np
import ml_dtypes

BF16 = ml_dtypes.bfloat16

NUM_VARS = 8
HID = 128
B = 8192
NCORES = 8
BC = B // NCORES  # 1024 batch per core
HALF = BC // 2  # 512: one fp32 PSUM bank / fp32 matmul free-dim limit

PAIRS = list(itertools.combinations(range(NUM_VARS), 2))  # 28
TRIPS = list(itertools.combinations(range(NUM_VARS), 3))  # 56
N1, N2, N3 = NUM_VARS, len(PAIRS), len(TRIPS)
NNETS = N1 + N2 + N3  # 92
CHUNK = 4  # nets per hidden-weight DMA chunk
NCHUNKS = NNETS // CHUNK  # 23
# Input-layer weights pack 3 nets per partition-block: matmul lhsT base
# partition must be 0/32/64, so K is padded 8 -> 32 with zero rows and
# x is replicated at partition bases 0/32/64.
WIN_K = 32
WIN_NPB = 3  # nets per partition-block (bases 0, 32, 64)
WIN_BLOCKS = (NNETS + WIN_NPB - 1) // WIN_NPB  # 31 column blocks

_CACHE = {}


def _coeffs():
    """Exact linear expansion of the HDMR combination.

    Basis: [g1_0..7, g2_0..27, g3_0..55, f0] (93 components).  Returns
    (c[92], c_f0) such that final = sum_n c_n g_n + c_f0 * f0.
    Note the reference indexes f_jj by *variable* index (0..7), not pair
    index -- reproduced faithfully.
    """
    dim = NNETS + 1
    e = np.eye(dim, dtype=np.float64)
    f0v = e[NNETS]
    f1 = [e[j] - f0v for j in range(N1)]
    f2 = [e[N1 + p] - f1[a] - f1[b] - f0v for p, (a, b) in enumerate(PAIRS)]
    f3 = [
        e[N1 + N2 + t] - f2[i] - f2[j] - f2[k] - f1[i] - f1[j] - f1[k] - f0v
        for t, (i, j, k) in enumerate(TRIPS)
    ]
    final = f0v + sum(f1) + sum(f2) + sum(f3)
    return final[:NNETS], final[NNETS]


def _net_vars():
    """Variable tuple per net, in net order (singles, pairs, trips)."""
    return [(j,) for j in range(N1)] + PAIRS + TRIPS


def _build_bass():
    from concourse import tile
    from concourse.bacc import Bacc
    import concourse.mybir as mybir

    f32 = mybir.dt.float32
    f32r = mybir.dt.float32r
    SIG = mybir.ActivationFunctionType.Sigmoid
    IDENT = mybir.ActivationFunctionType.Identity

    nc = Bacc(
        "TRN2",
        target_bir_lowering=False,
        debug=False,
        enable_asserts=False,
        num_devices=1,
    )

    bf16 = mybir.dt.bfloat16
    # x replicated at partition bases 0/32/64 (rows 8..31 of each block zero).
    # Input layer runs in bf16: halves the startup-critical DMAs and the
    # first matmuls stream in 1 pass; error contribution ~5e-5, negligible
    # against the ~2e-3 sigmoid-spline floor.
    xT_d = nc.dram_tensor("xT", [WIN_NPB * WIN_K, BC], bf16, kind="ExternalInput")
    # w_in packed: net n at partition base 32*(n%3), col block n//3
    w_in_d = nc.dram_tensor(
        "w_in", [WIN_NPB * WIN_K, WIN_BLOCKS * HID], bf16, kind="ExternalInput"
    )
    b_in_d = nc.dram_tensor("b_in", [HID, NNETS], f32, kind="ExternalInput")
    w_h_d = nc.dram_tensor("w_h", [HID, NNETS * 2 * HID], f32r, kind="ExternalInput")
    b_h_d = nc.dram_tensor("b_h", [HID, 2 * NNETS], f32, kind="ExternalInput")
    w_out_d = nc.dram_tensor("w_out", [HID, NNETS], f32r, kind="ExternalInput")
    cb_d = nc.dram_tensor("cb", [1, 1], f32, kind="ExternalInput")
    out_d = nc.dram_tensor("out", [1, BC], f32, kind="ExternalOutput")

    with tile.TileContext(nc) as tc:
        with ExitStack() as ctx:
            const = ctx.enter_context(tc.tile_pool(name="const", bufs=1))

            # Warm the sigmoid table at t=0 so the ~2.7us ACT table load
            # overlaps the initial weight DMAs instead of serializing after
            # them.  memset has no deps, so the dummy sigmoid issues first.
            warm = const.tile([1, 2], f32, tag="warm", name="warm_sb")
            nc.gpsimd.memset(warm[:, 0:1], 0.0)
            nc.scalar.activation(warm[:, 1:2], warm[:, 0:1], SIG)

            # DMA issue order = first-use order: triggers serialize at
            # ~625ns apiece, so net 0's dependencies go first.
            xT_sb = const.tile([WIN_NPB * WIN_K, BC], bf16, tag="xT", name="xT_sb")
            nc.sync.dma_start(xT_sb[:], xT_d.ap())

            # w_in in 4 separate chunk tiles (8 column-blocks each) so net
            # 0's input matmul waits on ~380KB, not the full 1.5MB.
            wi_cw = 8 * HID
            wi_tiles = []
            for ci in range(4):
                lo = ci * wi_cw
                hi = min((ci + 1) * wi_cw, WIN_BLOCKS * HID)
                t = const.tile(
                    [WIN_NPB * WIN_K, hi - lo], bf16, tag=f"wi{ci}", name=f"wi{ci}"
                )
                wi_tiles.append(t)
            nc.sync.dma_start(wi_tiles[0][:], w_in_d.ap()[:, 0:wi_cw])

            b_in_sb = const.tile([HID, NNETS], f32, tag="b_in", name="b_in_sb")
            nc.sync.dma_start(b_in_sb[:], b_in_d.ap())

            # Hidden weights in per-4-net chunks so net 0 starts without
            # waiting for the full 12 MB.
            wh_tiles = []
            cw = CHUNK * 2 * HID
            for ci in range(NCHUNKS):
                t = const.tile([HID, cw], f32r, tag=f"wh{ci}", name=f"wh{ci}")
                wh_tiles.append(t)
            nc.sync.dma_start(wh_tiles[0][:], w_h_d.ap()[:, 0:cw])

            b_h_sb = const.tile([HID, 2 * NNETS], f32, tag="b_h", name="b_h_sb")
            nc.sync.dma_start(b_h_sb[:], b_h_d.ap())
            w_out_sb = const.tile([HID, NNETS], f32r, tag="w_out", name="w_out_sb")
            nc.sync.dma_start(w_out_sb[:], w_out_d.ap())
            cb_sb = const.tile([1, 1], f32, tag="cb", name="cb_sb")
            nc.sync.dma_start(cb_sb[:], cb_d.ap())

            for ci in range(1, NCHUNKS):
                nc.sync.dma_start(
                    wh_tiles[ci][:], w_h_d.ap()[:, ci * cw : (ci + 1) * cw]
                )
                if ci < 4:
                    lo = ci * wi_cw
                    hi = min((ci + 1) * wi_cw, WIN_BLOCKS * HID)
                    nc.sync.dma_start(wi_tiles[ci][:], w_in_d.ap()[:, lo:hi])

            ps_in = ctx.enter_context(tc.tile_pool(name="ps_in", bufs=1, space="PSUM"))
            ps_h1 = ctx.enter_context(tc.tile_pool(name="ps_h1", bufs=1, space="PSUM"))
            ps_h2 = ctx.enter_context(tc.tile_pool(name="ps_h2", bufs=1, space="PSUM"))
            ps_acc = ctx.enter_context(
                tc.tile_pool(name="ps_acc", bufs=1, space="PSUM")
            )
            sb_hin = ctx.enter_context(tc.tile_pool(name="sb_hin", bufs=2))
            sb_h1 = ctx.enter_context(tc.tile_pool(name="sb_h1", bufs=2))
            sb_h2 = ctx.enter_context(tc.tile_pool(name="sb_h2", bufs=2))

            acc = ps_acc.tile([1, BC], f32, tag="acc", name="acc")

            halves = [(0, HALF), (HALF, BC)]
            for n in range(NNETS):
                ci, lo = divmod(n, CHUNK)
                wh = wh_tiles[ci]
                cblk, j = divmod(n, WIN_NPB)  # col block, partition base 32*j
                wic, wir = divmod(cblk, 8)  # w_in chunk tile, block within
                win = wi_tiles[wic][
                    j * WIN_K : (j + 1) * WIN_K, wir * HID : (wir + 1) * HID
                ]

                in_ps = ps_in.tile([HID, BC], f32, tag="in_ps", name=f"in_ps{n}")
                for a, b in halves:
                    nc.tensor.matmul(
                        in_ps[:, a:b],
                        win,
                        xT_sb[j * WIN_K : (j + 1) * WIN_K, a:b],
                        start=True,
                        stop=True,
                    )
                hin = sb_hin.tile([HID, BC], f32r, tag="hin", name=f"hin{n}")
                nc.scalar.activation(
                    hin[:], in_ps[:], SIG, bias=b_in_sb[:, n : n + 1]
                )

                h1_ps = ps_h1.tile([HID, BC], f32, tag="h1_ps", name=f"h1_ps{n}")
                for a, b in halves:
                    nc.tensor.matmul(
                        h1_ps[:, a:b],
                        wh[:, (lo * 2 + 0) * HID : (lo * 2 + 1) * HID],
                        hin[:, a:b],
                        start=True,
                        stop=True,
                    )
                h1 = sb_h1.tile([HID, BC], f32r, tag="h1", name=f"h1_{n}")
                nc.scalar.activation(
                    h1[:], h1_ps[:], SIG, bias=b_h_sb[:, 2 * n : 2 * n + 1]
                )

                h2_ps = ps_h2.tile([HID, BC], f32, tag="h2_ps", name=f"h2_ps{n}")
                for a, b in halves:
                    nc.tensor.matmul(
                        h2_ps[:, a:b],
                        wh[:, (lo * 2 + 1) * HID : (lo * 2 + 2) * HID],
                        h1[:, a:b],
                        start=True,
                        stop=True,
                    )
                h2 = sb_h2.tile([HID, BC], f32r, tag="h2", name=f"h2_{n}")
                nc.scalar.activation(
                    h2[:], h2_ps[:], SIG, bias=b_h_sb[:, 2 * n + 1 : 2 * n + 2]
                )

                for a, b in halves:
                    nc.tensor.matmul(
                        acc[:, a:b],
                        w_out_sb[:, n : n + 1],
                        h2[:, a:b],
                        start=(n == 0),
                        stop=(n == NNETS - 1),
                    )

            out_sb = const.tile([1, BC], f32, tag="out_sb", name="out_sb")
            nc.scalar.activation(out_sb[:], acc[:], IDENT, bias=cb_sb[:])
            nc.sync.dma_start(out_d.ap(), out_sb[:])

    nc.finalize()
    return nc


def _prep_weights(inputs):
    c, c_f0 = _coeffs()
    nets = _net_vars()

    groups = []
    for tag, count in (("1", N1), ("2", N2), ("3", N3)):
        groups.append(
            dict(
                W_in=np.asarray(inputs[f"W_in_{tag}"], np.float32),
                b_in=np.asarray(inputs[f"b_in_{tag}"], np.float32),
                W_h=np.asarray(inputs[f"W_h_{tag}"], np.float32),
                b_h=np.asarray(inputs[f"b_h_{tag}"], np.float32),
                W_out=np.asarray(inputs[f"W_out_{tag}"], np.float32),
                b_out=np.asarray(inputs[f"b_out_{tag}"], np.float32),
                n=count,
            )
        )

    w_in = np.zeros((WIN_NPB * WIN_K, WIN_BLOCKS * HID), np.float32)
    b_in = np.zeros((HID, NNETS), np.float32)
    w_h = np.zeros((HID, NNETS * 2 * HID), np.float32)
    b_h = np.zeros((HID, 2 * NNETS), np.float32)
    w_out = np.zeros((HID, NNETS), np.float32)
    cb = np.float64(c_f0) * np.float64(inputs["f0"])

    n = 0
    for g in groups:
        for k in range(g["n"]):
            vars_n = nets[n]
            cblk, j = divmod(n, WIN_NPB)
            for i, v in enumerate(vars_n):
                w_in[j * WIN_K + v, cblk * HID : (cblk + 1) * HID] = g["W_in"][k, :, i]
            b_in[:, n] = g["b_in"][k]
            for l in range(2):
                w_h[:, (n * 2 + l) * HID : (n * 2 + l + 1) * HID] = g["W_h"][k, l].T
                b_h[:, 2 * n + l] = g["b_h"][k, l]
            w_out[:, n] = c[n] * g["W_out"][k, 0, :]
            cb += np.float64(c[n]) * np.float64(g["b_out"][k])
            n += 1
    assert n == NNETS

    return dict(
        w_in=w_in.astype(BF16),
        b_in=b_in,
        w_h=w_h,
        b_h=b_h,
        w_out=w_out,
        cb=np.array([[cb]], np.float32),
    )


def make_in_maps(inputs):
    w = _prep_weights(inputs)
    x = np.asarray(inputs["x"], np.float32)
    xT = np.zeros((WIN_NPB * WIN_K, B), np.float32)
    for j in range(WIN_NPB):
        xT[j * WIN_K : j * WIN_K + NUM_VARS] = x.T
    xT = xT.astype(BF16)
    in_maps = []
    for core in range(NCORES):
        m = dict(w)
        m["xT"] = np.ascontiguousarray(xT[:, core * BC : (core + 1) * BC])
        in_maps.append(m)
    return in_maps


def kernel(**inputs):
    from concourse.bass_utils import run_bass_kernel_spmd

    if "nc" not in _CACHE:
        _CACHE["nc"] = _build_bass()
    nc = _CACHE["nc"]

    in_maps = make_in_maps(inputs)
    res = run_bass_kernel_spmd(nc, in_maps, core_ids=list(range(NCORES)))
    out = np.concatenate([r["out"].reshape(-1) for r in res.results])
    return out.astype(np.float32)[:, None]

